# revision 1
# baseline (speedup 1.0000x reference)
"""Multi-head attention (B=2, S=2048, D=1024, H=16) on 8 NeuronCores.

Sharding: tensor-parallel over heads — 2 heads per core. Each core computes
q/k/v projections for its 128 output columns, full attention for its 2 heads
(both batches), and a partial out-projection [4096, 1024]. Host sums the 8
partials and adds the output bias.

Device-side layout choices:
  - Q and K are produced *transposed* ([head_cols, tokens]) straight out of
    the projection matmuls — the operand layout the scores^T matmul wants
    (contract dim = head dim = 64).
  - scores are computed transposed ([keys, q]) so exp applies elementwise and
    attn@V contracts keys on the partition dim — the big P matrix is never
    transposed.
  - V carries an extra all-ones column so attn@[V|1] yields the softmax
    denominator (row 64) along with the unnormalized output (rows 0..63).
  - softmax skips max-subtraction: scores are ~N(0, 0.33^2) by construction.
  - exp runs on 1024-wide tiles (amortizes ACT's ~352-cycle fixed cost); ACT
    does nothing but exp except the final batch's output staging.
  - exp'd scores live in per-(b,h,q-halfgroup) tiles, double-buffered, so the
    scores->exp->attn@V pipeline flows across heads/batches.
  - softmax denominators: DVE reciprocal -> GpSimd partition_broadcast (idle
    engine) -> DVE multiply during PSUM evacuation. PE never blocks on it.
"""

import os
import numpy as np
import ml_dtypes

B, S, D, H = 2, 2048, 1024, 16
HD = D // H          # 64
BS = B * S           # 4096 tokens
NCORES = 8
HPC = H // NCORES    # heads per core = 2
CPC = HPC * HD       # output cols per core = 128
KC = D // 128        # contract chunks = 8
QCH = 512            # matmul moving free dim
NKT = S // 128       # 16 key tiles per batch
QG = 1024            # q-group width (exp tile / et tile width)
NQG = S // QG        # 2 q-groups per batch

BF16 = ml_dtypes.bfloat16

_prog = None


def _build_program():
    import concourse.bacc as bacc
    import concourse.tile as tile
    from concourse import mybir

    f32 = mybir.dt.float32
    bf16 = mybir.dt.bfloat16
    AF = mybir.ActivationFunctionType

    nc = bacc.Bacc("TRN2", debug=False, enable_asserts=False, num_devices=NCORES)

    xT = nc.dram_tensor("xT", [D, BS], bf16, kind="ExternalInput").ap()
    wq = nc.dram_tensor("wq", [D, CPC], bf16, kind="ExternalInput").ap()
    wk = nc.dram_tensor("wk", [D, CPC], bf16, kind="ExternalInput").ap()
    wv = nc.dram_tensor("wv", [D, CPC], bf16, kind="ExternalInput").ap()
    wo = nc.dram_tensor("wo", [CPC, D], bf16, kind="ExternalInput").ap()
    bq = nc.dram_tensor("bq", [CPC, 1], f32, kind="ExternalInput").ap()
    bk = nc.dram_tensor("bk", [CPC, 1], f32, kind="ExternalInput").ap()
    bv = nc.dram_tensor("bv", [1, CPC], bf16, kind="ExternalInput").ap()
    out = nc.dram_tensor("out", [BS, D], f32, kind="ExternalOutput").ap()

    SCALE = float(1.0 / np.sqrt(HD))

    with tile.TileContext(nc) as tc:
        with (
            tc.tile_pool(name="big", bufs=1) as big,
            tc.tile_pool(name="sm", bufs=1) as sm,
            tc.tile_pool(name="attn", bufs=2) as attn,
            tc.tile_pool(name="etp", bufs=2) as etp,
            tc.tile_pool(name="ostage", bufs=4) as ostage,
            tc.tile_pool(name="ps", bufs=2, space="PSUM") as ps,
        ):
            # ---- resident SBUF tensors ----
            xt_sb = big.tile([128, KC, BS], bf16, name="xt_sb", tag="xt")
            qt_sb = big.tile([128, BS], bf16, name="qt_sb", tag="qt")
            kt_sb = big.tile([128, BS], bf16, name="kt_sb", tag="kt")
            # V|ones per head: [keys(128) x keytile(32) x (64 V + 1 ones)*2]
            v_sb = big.tile([128, B * NKT, 2 * (HD + 1)], bf16, name="v_sb", tag="v")
            wo_sb = big.tile([128, D], bf16, name="wo_sb", tag="wo")

            wq_sb = sm.tile([128, KC, CPC], bf16, name="wq_sb", tag="wq")
            wk_sb = sm.tile([128, KC, CPC], bf16, name="wk_sb", tag="wk")
            wv_sb = sm.tile([128, KC, CPC], bf16, name="wv_sb", tag="wv")
            bq_sb = sm.tile([CPC, 1], f32, name="bq_sb", tag="bq")
            bk_sb = sm.tile([CPC, 1], f32, name="bk_sb", tag="bk")
            bv_sb = sm.tile([1, CPC], bf16, name="bv_sb", tag="bv")
            ones_bf = sm.tile([1, 128], bf16, name="ones_bf", tag="onesb")

            nc.vector.memset(ones_bf, 1.0)
            nc.vector.memset(v_sb[:, :, HD : HD + 1], 1.0)
            nc.vector.memset(v_sb[:, :, 2 * HD + 1 : 2 * HD + 2], 1.0)

            # first QK-proj tile needs only wq/wk + token-block 0 of xT:
            # emit those DMAs first so PE starts ASAP
            xt_r = xT.rearrange("(c p) n -> p c n", p=128)
            nc.sync.dma_start(out=wq_sb, in_=wq.rearrange("(c p) n -> p c n", p=128))
            nc.sync.dma_start(out=wk_sb, in_=wk.rearrange("(c p) n -> p c n", p=128))
            for c in range(KC):
                nc.sync.dma_start(out=xt_sb[:, c, 0:1024], in_=xt_r[:, c, 0:1024])
            nc.sync.dma_start(out=wv_sb, in_=wv.rearrange("(c p) n -> p c n", p=128))
            nc.sync.dma_start(out=wo_sb, in_=wo)
            nc.sync.dma_start(out=bq_sb, in_=bq)
            nc.sync.dma_start(out=bk_sb, in_=bk)
            nc.sync.dma_start(out=bv_sb, in_=bv)
            for tb in range(1, BS // 1024):
                for c in range(KC):
                    nc.sync.dma_start(
                        out=xt_sb[:, c, tb * 1024 : (tb + 1) * 1024],
                        in_=xt_r[:, c, tb * 1024 : (tb + 1) * 1024],
                    )

            # ---- Q^T / K^T projections: [head_cols(128), tokens] ----
            for name, w_sb, b_sb, dst in (
                ("q", wq_sb, bq_sb, qt_sb),
                ("k", wk_sb, bk_sb, kt_sb),
            ):
                for t in range(BS // 1024):
                    pp = ps.tile([128, 1024], f32, name=f"pp_{name}{t}", tag="sp")
                    for c in range(KC):
                        for half in range(2):
                            nc.tensor.matmul(
                                pp[:, half * QCH : (half + 1) * QCH],
                                lhsT=w_sb[:, c, :],
                                rhs=xt_sb[:, c, t * 1024 + half * QCH : t * 1024 + (half + 1) * QCH],
                                start=(c == 0),
                                stop=(c == KC - 1),
                            )
                    nc.vector.tensor_scalar_add(
                        dst[:, t * 1024 : (t + 1) * 1024], pp, b_sb
                    )

            # ---- V projection (+bias via rank-1 matmul): natural [keys, cols] ----
            for kt in range(B * NKT):
                vp = ps.tile([128, CPC], f32, name=f"vp{kt}", tag="small", bufs=4)
                for c in range(KC):
                    nc.tensor.matmul(
                        vp,
                        lhsT=xt_sb[:, c, kt * 128 : (kt + 1) * 128],
                        rhs=wv_sb[:, c, :],
                        start=(c == 0),
                        stop=False,
                    )
                nc.tensor.matmul(vp, lhsT=ones_bf, rhs=bv_sb, start=False, stop=True)
                nc.vector.tensor_copy(
                    v_sb[:, kt, :].rearrange("p (h c) -> p h c", h=2)[:, :, 0:HD],
                    vp.rearrange("p (h c) -> p h c", h=2),
                )

            # ---- attention ----
            for b in range(B):
                ot_sb = attn.tile([128, S], bf16, name=f"ot{b}", tag="ot")
                for h in range(HPC):
                    hp = h * HD
                    for qg in range(NQG):
                        q0 = b * S + qg * QG
                        et = etp.tile([128, NKT, QG], bf16, name=f"et{b}{h}{qg}", tag="et")
                        for kt in range(NKT):
                            sp = ps.tile([128, QG], f32, name=f"sp{b}{h}{qg}{kt}", tag="sp")
                            for qh in range(2):
                                nc.tensor.matmul(
                                    sp[:, qh * QCH : (qh + 1) * QCH],
                                    lhsT=kt_sb[hp : hp + HD, b * S + kt * 128 : b * S + (kt + 1) * 128],
                                    rhs=qt_sb[hp : hp + HD, q0 + qh * QCH : q0 + (qh + 1) * QCH],
                                    start=True,
                                    stop=True,
                                )
                            nc.scalar.activation(et[:, kt, :], sp, AF.Exp, scale=SCALE)
                        # o^T_unnorm + sumexp; denominators off the PE path
                        ops = []
                        for qc in range(QG // QCH):
                            op = ps.tile([HD + 1, QCH], f32, name=f"op{b}{h}{qg}{qc}", tag="small", bufs=4)
                            for kt in range(NKT):
                                nc.tensor.matmul(
                                    op,
                                    lhsT=v_sb[:, b * NKT + kt, h * (HD + 1) : (h + 1) * (HD + 1)],
                                    rhs=et[:, kt, qc * QCH : (qc + 1) * QCH],
                                    start=(kt == 0),
                                    stop=(kt == NKT - 1),
                                )
                            ops.append(op)
                        for qc, op in enumerate(ops):
                            rc = ostage.tile([1, QCH], f32, name=f"rc{b}{h}{qg}{qc}", tag="rc")
                            nc.vector.reciprocal(rc, op[HD : HD + 1, :])
                            rbs = ostage.tile([HD, QCH], f32, name=f"rbs{b}{h}{qg}{qc}", tag="rbs")
                            nc.gpsimd.partition_broadcast(rbs, rc)
                            nc.vector.tensor_mul(
                                ot_sb[hp : hp + HD, qg * QG + qc * QCH : qg * QG + (qc + 1) * QCH],
                                op[0:HD, :],
                                rbs,
                            )
                # ---- partial out-projection for batch b ----
                tail = b == B - 1
                for qt in range(S // 128):
                    os_ = ostage.tile([128, 1024], f32, name=f"os{b}{qt}", tag="os", bufs=3)
                    for nh in range(2):
                        pq = ps.tile([128, QCH], f32, name=f"pq{b}{qt}{nh}", tag="small", bufs=4)
                        nc.tensor.matmul(
                            pq,
                            lhsT=ot_sb[:, qt * 128 : (qt + 1) * 128],
                            rhs=wo_sb[:, nh * QCH : (nh + 1) * QCH],
                            start=True,
                            stop=True,
                        )
                        if tail and qt >= S // 256:
                            nc.scalar.copy(os_[:, nh * QCH : (nh + 1) * QCH], pq)
                        else:
                            nc.vector.tensor_copy(os_[:, nh * QCH : (nh + 1) * QCH], pq)
                    nc.sync.dma_start(
                        out=out[b * S + qt * 128 : b * S + (qt + 1) * 128, :],
                        in_=os_,
                    )

    nc.compile()
    return nc


def _get_prog():
    global _prog
    if _prog is None:
        _prog = _build_program()
    return _prog


def kernel(x, Wq, bq, Wk, bk, Wv, bv, Wo, bo):
    from concourse import bass_utils

    nc = _get_prog()

    xT = np.ascontiguousarray(
        np.asarray(x, dtype=np.float32).reshape(BS, D).T
    ).astype(BF16)

    in_maps = []
    for c in range(NCORES):
        cols = slice(c * CPC, (c + 1) * CPC)
        in_maps.append(
            {
                "xT": xT,
                "wq": np.ascontiguousarray(Wq[cols, :].T).astype(BF16),
                "wk": np.ascontiguousarray(Wk[cols, :].T).astype(BF16),
                "wv": np.ascontiguousarray(Wv[cols, :].T).astype(BF16),
                "wo": np.ascontiguousarray(Wo[:, cols].T).astype(BF16),
                "bq": np.asarray(bq[cols], np.float32).reshape(CPC, 1),
                "bk": np.asarray(bk[cols], np.float32).reshape(CPC, 1),
                "bv": np.asarray(bv[cols], np.float32).reshape(1, CPC).astype(BF16),
            }
        )

    res = bass_utils.run_bass_kernel_spmd(
        nc,
        in_maps,
        core_ids=list(range(NCORES)),
        trace=bool(int(os.environ.get("KERNEL_TRACE", "0"))),
    )
    kernel.last_results = res

    acc = np.zeros((BS, D), np.float64)
    for c in range(NCORES):
        acc += res.results[c]["out"].astype(np.float64)
    acc += np.asarray(bo, np.float64)[None, :]
    return acc.reshape(B, S, D).astype(np.float32)



# revision 2
# speedup vs baseline: 1.1477x; 1.1477x over previous
"""Multi-head attention (B=2, S=2048, D=1024, H=16) on 8 NeuronCores.

Sharding: tensor-parallel over heads - 2 heads per core. Each core computes
q/k/v projections for its 128 output columns, full attention for its 2 heads
(both batches), and a partial out-projection [4096, 1024] in bf16. Host sums
the 8 partials (fp64) and adds the output bias.

v2 design notes (vs v1):
  - Scores matmuls (contract = head dim = 64) run as 64x128 row-tiled PE
    pairs: head 0 on array rows 0-63 (tile_position (0,0)), head 1 on rows
    64-127 ((64,0)) concurrently -> ~2x scores throughput. K^T/Q^T layouts
    already place head h's data on SBUF partitions 64h..64h+63.
  - Attention is a per-key-tile software pipeline: scores(g,kt) -> exp(g,kt)
    on ACT -> attnV(g-1,kt), so the ACT engine (the 147us exp floor: 16.8M
    exps at 1 elem/cycle/lane) stays saturated while the PE fills its slack
    with the next batch's projections and the previous group's out-proj.
  - Scores psum tiles pair both heads [128, 2, 512] so one ACT instruction
    exps 1024 elements/partition (amortizes the ~352-cycle ACT overhead).
  - Softmax denominators (row 64 of the attnV psum, via the V|ones trick)
    are collected per-group into a [1, 2, 512] fp32 tile and inverted with
    ONE reciprocal_approx_fast - the v1 per-chunk [1,512] nc.vector.reciprocal
    calls (8 cyc/elem iterative divide) burned 53us of DVE and stalled the
    PE long enough to re-throttle the HAM clock gate (~90us at half clock).
  - One gpsimd partition_broadcast per group ([64, 2, 512]) replicates the
    reciprocals; two DVE muls normalize straight out of psum into ot_sb.
  - Out-projection of group g runs during group g+1's attention; partials
    are written to HBM as bf16 (halves output DMA).
"""

import os
import numpy as np
import ml_dtypes

B, S, D, H = 2, 2048, 1024, 16
HD = D // H          # 64
BS = B * S           # 4096 tokens
NCORES = 8
HPC = H // NCORES    # heads per core = 2
CPC = HPC * HD       # output cols per core = 128
KC = D // 128        # contract chunks = 8
NKT = S // 128       # 16 key tiles per batch
QG = 512             # q-group width (one psum bank of fp32)
NQG = S // QG        # 4 q-groups per batch

BF16 = ml_dtypes.bfloat16

_prog = None


def _build_program():
    import concourse.bacc as bacc
    import concourse.tile as tile
    from concourse import mybir

    f32 = mybir.dt.float32
    bf16 = mybir.dt.bfloat16
    AF = mybir.ActivationFunctionType

    nc = bacc.Bacc("TRN2", debug=False, enable_asserts=False, num_devices=NCORES)

    xT = nc.dram_tensor("xT", [D, BS], bf16, kind="ExternalInput").ap()
    wq = nc.dram_tensor("wq", [D, CPC], bf16, kind="ExternalInput").ap()
    wk = nc.dram_tensor("wk", [D, CPC], bf16, kind="ExternalInput").ap()
    wv = nc.dram_tensor("wv", [D, CPC], bf16, kind="ExternalInput").ap()
    wo = nc.dram_tensor("wo", [CPC, D], bf16, kind="ExternalInput").ap()
    bq = nc.dram_tensor("bq", [CPC, 1], f32, kind="ExternalInput").ap()
    bk = nc.dram_tensor("bk", [CPC, 1], f32, kind="ExternalInput").ap()
    bv = nc.dram_tensor("bv", [1, CPC], bf16, kind="ExternalInput").ap()
    out = nc.dram_tensor("out", [BS, D], bf16, kind="ExternalOutput").ap()

    SCALE = float(1.0 / np.sqrt(HD))

    with tile.TileContext(nc) as tc:
        with (
            tc.tile_pool(name="big", bufs=1) as big,
            tc.tile_pool(name="sm", bufs=1) as sm,
            tc.tile_pool(name="attn", bufs=2) as attn,
            tc.tile_pool(name="etp", bufs=24) as etp,
            tc.tile_pool(name="nrm", bufs=2) as nrm,
            tc.tile_pool(name="ostage", bufs=3) as ostage,
            tc.tile_pool(name="ps", bufs=2, space="PSUM") as ps,
        ):
            # ---- resident SBUF tensors ----
            xt_sb = big.tile([128, KC, BS], bf16, name="xt_sb", tag="xt")
            qt_sb = big.tile([128, BS], bf16, name="qt_sb", tag="qt")
            kt_sb = big.tile([128, BS], bf16, name="kt_sb", tag="kt")
            # V|ones per head: [keys(128) x keytile(32) x (64 V + 1 ones)*2]
            v_sb = big.tile([128, B * NKT, 2 * (HD + 1)], bf16, name="v_sb", tag="v")
            wo_sb = big.tile([128, D], bf16, name="wo_sb", tag="wo")

            wq_sb = sm.tile([128, KC, CPC], bf16, name="wq_sb", tag="wq")
            wk_sb = sm.tile([128, KC, CPC], bf16, name="wk_sb", tag="wk")
            wv_sb = sm.tile([128, KC, CPC], bf16, name="wv_sb", tag="wv")
            bq_sb = sm.tile([CPC, 1], f32, name="bq_sb", tag="bq")
            bk_sb = sm.tile([CPC, 1], f32, name="bk_sb", tag="bk")
            bv_sb = sm.tile([1, CPC], bf16, name="bv_sb", tag="bv")
            ones_bf = sm.tile([1, 128], bf16, name="ones_bf", tag="onesb")

            nc.vector.memset(ones_bf, 1.0)
            nc.vector.memset(v_sb[:, :, HD : HD + 1], 1.0)
            nc.vector.memset(v_sb[:, :, 2 * HD + 1 : 2 * HD + 2], 1.0)

            # first QK-proj tile needs only wq/wk + token-block 0 of xT:
            # emit those DMAs first so PE starts ASAP
            xt_r = xT.rearrange("(c p) n -> p c n", p=128)
            nc.sync.dma_start(out=wq_sb, in_=wq.rearrange("(c p) n -> p c n", p=128))
            nc.sync.dma_start(out=wk_sb, in_=wk.rearrange("(c p) n -> p c n", p=128))
            for c in range(KC):
                nc.sync.dma_start(out=xt_sb[:, c, 0:1024], in_=xt_r[:, c, 0:1024])
            nc.sync.dma_start(out=bq_sb, in_=bq)
            nc.sync.dma_start(out=bk_sb, in_=bk)
            nc.sync.dma_start(out=wv_sb, in_=wv.rearrange("(c p) n -> p c n", p=128))
            nc.sync.dma_start(out=bv_sb, in_=bv)
            for tb in range(1, BS // 1024):
                for c in range(KC):
                    nc.sync.dma_start(
                        out=xt_sb[:, c, tb * 1024 : (tb + 1) * 1024],
                        in_=xt_r[:, c, tb * 1024 : (tb + 1) * 1024],
                    )
            nc.sync.dma_start(out=wo_sb, in_=wo)

            # ---- Q^T / K^T projection for one 1024-token block ----
            def qk_proj(t):
                for name, w_sb, b_sb, dst in (
                    ("q", wq_sb, bq_sb, qt_sb),
                    ("k", wk_sb, bk_sb, kt_sb),
                ):
                    pp = ps.tile([128, 1024], f32, name=f"pp_{name}{t}", tag="sp")
                    for c in range(KC):
                        for half in range(2):
                            nc.tensor.matmul(
                                pp[:, half * 512 : (half + 1) * 512],
                                lhsT=w_sb[:, c, :],
                                rhs=xt_sb[:, c, t * 1024 + half * 512 : t * 1024 + (half + 1) * 512],
                                start=(c == 0),
                                stop=(c == KC - 1),
                            )
                    nc.vector.tensor_scalar_add(
                        dst[:, t * 1024 : (t + 1) * 1024], pp, b_sb
                    )

            # ---- V projection (+bias via rank-1 matmul) for one key tile ----
            def v_proj(kt):
                vp = ps.tile([128, CPC], f32, name=f"vp{kt}", tag="op", bufs=4)
                for c in range(KC):
                    nc.tensor.matmul(
                        vp,
                        lhsT=xt_sb[:, c, kt * 128 : (kt + 1) * 128],
                        rhs=wv_sb[:, c, :],
                        start=(c == 0),
                        stop=False,
                    )
                nc.tensor.matmul(vp, lhsT=ones_bf, rhs=bv_sb, start=False, stop=True)
                nc.vector.tensor_copy(
                    v_sb[:, kt, :].rearrange("p (h c) -> p h c", h=2)[:, :, 0:HD],
                    vp.rearrange("p (h c) -> p h c", h=2),
                )

            qk_proj(0)
            qk_proj(1)
            for kt in range(NKT):
                v_proj(kt)

            # ---- attention pipeline over groups g = (b, qg) ----
            # per group: 16 kt iterations of [scores pair -> exp -> attnV pair
            # of the previous group]; after the loop: normalize + out-project
            # the previous group. PE filler work (next batch's projections) is
            # emitted at group boundaries.
            groups = [(b, qg) for b in range(B) for qg in range(NQG)]
            NG = len(groups)

            et_tiles = {}   # (gi, kt) -> et tile [128, 2, QG]
            op_tiles = {}   # (gi, h) -> attnV psum tile [65, QG]
            ot_tiles = {}   # b -> ot_sb [128, S]

            def scores_pair(gi, kt):
                b, qg = groups[gi]
                q0 = b * S + qg * QG
                k0 = b * S + kt * 128
                sp = ps.tile([128, 2, QG], f32, name=f"sp{gi}_{kt}", tag="sp")
                for h in range(HPC):
                    hp = h * HD
                    nc.tensor.matmul(
                        sp[:, h, :],
                        lhsT=kt_sb[hp : hp + HD, k0 : k0 + 128],
                        rhs=qt_sb[hp : hp + HD, q0 : q0 + QG],
                        start=True,
                        stop=True,
                        tile_position=(hp, 0),
                    )
                et = etp.tile([128, 2, QG], bf16, name=f"et{gi}_{kt}", tag="et")
                nc.scalar.activation(et, sp, AF.Exp, scale=SCALE)
                et_tiles[(gi, kt)] = et

            def attnv_pair(gi, kt):
                b, qg = groups[gi]
                et = et_tiles.pop((gi, kt))
                for h in range(HPC):
                    if kt == 0:
                        op_tiles[(gi, h)] = ps.tile(
                            [HD + 1, QG], f32, name=f"op{gi}_{h}", tag="op", bufs=4
                        )
                    nc.tensor.matmul(
                        op_tiles[(gi, h)],
                        lhsT=v_sb[:, b * NKT + kt, h * (HD + 1) : (h + 1) * (HD + 1)],
                        rhs=et[:, h, :],
                        start=(kt == 0),
                        stop=(kt == NKT - 1),
                    )

            def normalize(gi):
                b, qg = groups[gi]
                if qg == 0:
                    ot_tiles[b] = attn.tile([128, S], bf16, name=f"ot{b}", tag="ot")
                ot = ot_tiles[b]
                den = nrm.tile([1, 2, QG], f32, name=f"den{gi}", tag="den")
                for h in range(HPC):
                    nc.vector.tensor_copy(
                        den[0:1, h, :], op_tiles[(gi, h)][HD : HD + 1, :]
                    )
                rq = nrm.tile([1, 2, QG], f32, name=f"rq{gi}", tag="rq")
                nc.vector.reciprocal_approx_fast(rq, den)
                rbs = nrm.tile([HD, 2, QG], f32, name=f"rbs{gi}", tag="rbs")
                nc.gpsimd.partition_broadcast(rbs, rq)
                for h in range(HPC):
                    op = op_tiles.pop((gi, h))
                    nc.vector.tensor_mul(
                        ot[h * HD : (h + 1) * HD, qg * QG : (qg + 1) * QG],
                        op[0:HD, :],
                        rbs[:, h, :],
                    )

            def out_proj(gi):
                b, qg = groups[gi]
                ot = ot_tiles[b]
                for qt in range(qg * (QG // 128), (qg + 1) * (QG // 128)):
                    pq = ps.tile([128, 1024], f32, name=f"pq{gi}_{qt}", tag="sp")
                    for nh in range(2):
                        nc.tensor.matmul(
                            pq[:, nh * 512 : (nh + 1) * 512],
                            lhsT=ot[:, qt * 128 : (qt + 1) * 128],
                            rhs=wo_sb[:, nh * 512 : (nh + 1) * 512],
                            start=True,
                            stop=True,
                        )
                    os_ = ostage.tile([128, 1024], bf16, name=f"os{gi}_{qt}", tag="os")
                    nc.vector.tensor_copy(os_, pq)
                    nc.sync.dma_start(
                        out=out[b * S + qt * 128 : b * S + (qt + 1) * 128, :],
                        in_=os_,
                    )

            # PE filler work for batch 1, emitted at batch-0 group boundaries
            fillers = {
                0: lambda: qk_proj(2),
                1: lambda: (qk_proj(3), *[v_proj(NKT + kt) for kt in range(5)]),
                2: lambda: [v_proj(NKT + kt) for kt in range(5, 10)],
                3: lambda: [v_proj(NKT + kt) for kt in range(10, 16)],
            }

            for gi in range(NG):
                last = gi == NG - 1
                for kt in range(NKT):
                    scores_pair(gi, kt)
                    if gi > 0:
                        attnv_pair(gi - 1, kt)
                    if last:
                        attnv_pair(gi, kt)
                if gi - 1 >= 0:
                    normalize(gi - 1)
                    out_proj(gi - 1)
                if gi in fillers:
                    fillers[gi]()
            normalize(NG - 1)
            out_proj(NG - 1)

    nc.compile()
    return nc


def _get_prog():
    global _prog
    if _prog is None:
        _prog = _build_program()
    return _prog


def kernel(x, Wq, bq, Wk, bk, Wv, bv, Wo, bo):
    from concourse import bass_utils

    nc = _get_prog()

    xT = np.ascontiguousarray(
        np.asarray(x, dtype=np.float32).reshape(BS, D).T
    ).astype(BF16)

    in_maps = []
    for c in range(NCORES):
        cols = slice(c * CPC, (c + 1) * CPC)
        in_maps.append(
            {
                "xT": xT,
                "wq": np.ascontiguousarray(Wq[cols, :].T).astype(BF16),
                "wk": np.ascontiguousarray(Wk[cols, :].T).astype(BF16),
                "wv": np.ascontiguousarray(Wv[cols, :].T).astype(BF16),
                "wo": np.ascontiguousarray(Wo[:, cols].T).astype(BF16),
                "bq": np.asarray(bq[cols], np.float32).reshape(CPC, 1),
                "bk": np.asarray(bk[cols], np.float32).reshape(CPC, 1),
                "bv": np.asarray(bv[cols], np.float32).reshape(1, CPC).astype(BF16),
            }
        )

    res = bass_utils.run_bass_kernel_spmd(
        nc,
        in_maps,
        core_ids=list(range(NCORES)),
        trace=bool(int(os.environ.get("KERNEL_TRACE", "0"))),
    )
    kernel.last_results = res

    acc = np.zeros((BS, D), np.float64)
    for c in range(NCORES):
        acc += res.results[c]["out"].astype(np.float64)
    acc += np.asarray(bo, np.float64)[None, :]
    return acc.reshape(B, S, D).astype(np.float32)


# revision 3
# speedup vs baseline: 1.2567x; 1.0950x over previous
"""Multi-head attention (B=2, S=2048, D=1024, H=16) on 8 NeuronCores.

Sharding: tensor-parallel over heads - 2 heads per core. Each core computes
q/k/v projections for its 128 output columns, full attention for its 2 heads
(both batches), and a partial out-projection [4096, 1024] in bf16. Host sums
the 8 partials (fp64) and adds the output bias.

v3 design: one global software pipeline over 128 "slots" (8 groups x 16 key
tiles). Each slot emits, in PE program order:
  1. the scores pair for (group, kt): both heads as concurrent 64x128
     row-tiled matmuls (contract = head dim = 64; head h lives on SBUF
     partitions 64h..64h+63 in the transposed Q^T/K^T layouts),
  2. one ACT exp over both heads' scores psum [128, 2, 512] (1024
     elems/partition amortizes ACT's ~352-cycle fixed cost) - the ACT engine
     is the attention-phase floor (16.8M exps ~= 147us) and paces the kernel,
  3. the attnV pair trailing 4 slots behind (V|ones trick: psum row 64
     accumulates the softmax denominator),
  4. one "filler" unit popped from a queue: QK-projection half-blocks,
     V-projection key tiles, out-projection row-tiles of finished groups,
     and normalizations - this keeps the PE busy during the ACT-paced
     attention instead of serializing before/after it.

Softmax denominators are inverted per-group with a single [1, 2, 512]
reciprocal_approx_fast (the v1 per-chunk [1,512] nc.vector.reciprocal burned
53us of DVE at 8 cyc/elem and stalled the PE into HAM clock re-throttles),
broadcast once via gpsimd, and applied by two DVE muls straight out of psum.
"""

import os
import numpy as np
import ml_dtypes

B, S, D, H = 2, 2048, 1024, 16
HD = D // H          # 64
BS = B * S           # 4096 tokens
NCORES = 8
HPC = H // NCORES    # heads per core = 2
CPC = HPC * HD       # output cols per core = 128
KC = D // 128        # contract chunks = 8
NKT = S // 128       # 16 key tiles per batch
QG = 512             # q-group width (one psum bank of fp32)
NQG = S // QG        # 4 q-groups per batch
TRAIL = 4            # attnV trails scores by this many slots

BF16 = ml_dtypes.bfloat16

_prog = None


def _build_program():
    import concourse.bacc as bacc
    import concourse.tile as tile
    from concourse import mybir

    f32 = mybir.dt.float32
    bf16 = mybir.dt.bfloat16
    AF = mybir.ActivationFunctionType

    nc = bacc.Bacc("TRN2", debug=False, enable_asserts=False, num_devices=NCORES)

    xT = nc.dram_tensor("xT", [D, BS], bf16, kind="ExternalInput").ap()
    wq = nc.dram_tensor("wq", [D, CPC], bf16, kind="ExternalInput").ap()
    wk = nc.dram_tensor("wk", [D, CPC], bf16, kind="ExternalInput").ap()
    wv = nc.dram_tensor("wv", [D, CPC], bf16, kind="ExternalInput").ap()
    wo = nc.dram_tensor("wo", [CPC, D], bf16, kind="ExternalInput").ap()
    bq = nc.dram_tensor("bq", [CPC, 1], f32, kind="ExternalInput").ap()
    bk = nc.dram_tensor("bk", [CPC, 1], f32, kind="ExternalInput").ap()
    bv = nc.dram_tensor("bv", [1, CPC], bf16, kind="ExternalInput").ap()
    out = nc.dram_tensor("out", [BS, D], bf16, kind="ExternalOutput").ap()

    SCALE = float(1.0 / np.sqrt(HD))

    with tile.TileContext(nc) as tc:
        with (
            tc.tile_pool(name="big", bufs=1) as big,
            tc.tile_pool(name="sm", bufs=1) as sm,
            tc.tile_pool(name="attn", bufs=2) as attn,
            tc.tile_pool(name="etp", bufs=8) as etp,
            tc.tile_pool(name="nrm", bufs=2) as nrm,
            tc.tile_pool(name="ostage", bufs=3) as ostage,
            tc.tile_pool(name="ps", bufs=2, space="PSUM") as ps,
        ):
            # ---- resident SBUF tensors ----
            xt_sb = big.tile([128, KC, BS], bf16, name="xt_sb", tag="xt")
            qt_sb = big.tile([128, BS], bf16, name="qt_sb", tag="qt")
            kt_sb = big.tile([128, BS], bf16, name="kt_sb", tag="kt")
            # V|ones per head: [keys(128) x keytile(32) x (64 V + 1 ones)*2]
            v_sb = big.tile([128, B * NKT, 2 * (HD + 1)], bf16, name="v_sb", tag="v")
            wo_sb = big.tile([128, D], bf16, name="wo_sb", tag="wo")

            wq_sb = sm.tile([128, KC, CPC], bf16, name="wq_sb", tag="wq")
            wk_sb = sm.tile([128, KC, CPC], bf16, name="wk_sb", tag="wk")
            wv_sb = sm.tile([128, KC, CPC], bf16, name="wv_sb", tag="wv")
            bq_sb = sm.tile([CPC, 1], f32, name="bq_sb", tag="bq")
            bk_sb = sm.tile([CPC, 1], f32, name="bk_sb", tag="bk")
            bv_sb = sm.tile([1, CPC], bf16, name="bv_sb", tag="bv")
            ones_bf = sm.tile([1, 128], bf16, name="ones_bf", tag="onesb")

            nc.vector.memset(ones_bf, 1.0)
            nc.vector.memset(v_sb[:, :, HD : HD + 1], 1.0)
            nc.vector.memset(v_sb[:, :, 2 * HD + 1 : 2 * HD + 2], 1.0)

            # DMA order: first QK-proj unit needs wq/wk + token block 0
            xt_r = xT.rearrange("(c p) n -> p c n", p=128)
            nc.sync.dma_start(out=wq_sb, in_=wq.rearrange("(c p) n -> p c n", p=128))
            nc.sync.dma_start(out=wk_sb, in_=wk.rearrange("(c p) n -> p c n", p=128))
            for c in range(KC):
                nc.sync.dma_start(out=xt_sb[:, c, 0:1024], in_=xt_r[:, c, 0:1024])
            nc.sync.dma_start(out=bq_sb, in_=bq)
            nc.sync.dma_start(out=bk_sb, in_=bk)
            for c in range(KC):
                nc.sync.dma_start(out=xt_sb[:, c, 1024:2048], in_=xt_r[:, c, 1024:2048])
            nc.sync.dma_start(out=wv_sb, in_=wv.rearrange("(c p) n -> p c n", p=128))
            nc.sync.dma_start(out=bv_sb, in_=bv)
            for tb in range(2, BS // 1024):
                for c in range(KC):
                    nc.sync.dma_start(
                        out=xt_sb[:, c, tb * 1024 : (tb + 1) * 1024],
                        in_=xt_r[:, c, tb * 1024 : (tb + 1) * 1024],
                    )
            nc.sync.dma_start(out=wo_sb, in_=wo)

            groups = [(b, qg) for b in range(B) for qg in range(NQG)]
            NG = len(groups)

            et_tiles = {}   # (gi, kt) -> et tile [128, 2, QG]
            op_tiles = {}   # (gi, h) -> attnV psum tile [65, QG]
            ot_tiles = {}   # b -> ot_sb [128, S]

            # ---- filler units (each ~1-2us of PE work) ----
            def qk_unit(is_q, half):
                # projects Q^T or K^T for tokens [512*half, 512*(half+1))
                w_sb, b_sb, dst = (
                    (wq_sb, bq_sb, qt_sb) if is_q else (wk_sb, bk_sb, kt_sb)
                )
                tok = half * 512

                def run():
                    pp = ps.tile(
                        [128, 512], f32, name=f"pp{int(is_q)}_{half}", tag="op", bufs=4
                    )
                    for c in range(KC):
                        nc.tensor.matmul(
                            pp,
                            lhsT=w_sb[:, c, :],
                            rhs=xt_sb[:, c, tok : tok + 512],
                            start=(c == 0),
                            stop=(c == KC - 1),
                        )
                    nc.vector.tensor_scalar_add(dst[:, tok : tok + 512], pp, b_sb)

                return run

            def v_unit(kt):
                def run():
                    vp = ps.tile([128, CPC], f32, name=f"vp{kt}", tag="op", bufs=4)
                    for c in range(KC):
                        nc.tensor.matmul(
                            vp,
                            lhsT=xt_sb[:, c, kt * 128 : (kt + 1) * 128],
                            rhs=wv_sb[:, c, :],
                            start=(c == 0),
                            stop=False,
                        )
                    nc.tensor.matmul(
                        vp, lhsT=ones_bf, rhs=bv_sb, start=False, stop=True
                    )
                    nc.vector.tensor_copy(
                        v_sb[:, kt, :].rearrange("p (h c) -> p h c", h=2)[:, :, 0:HD],
                        vp.rearrange("p (h c) -> p h c", h=2),
                    )

                return run

            def norm_unit(gi):
                b, qg = groups[gi]

                def run():
                    if qg == 0:
                        ot_tiles[b] = attn.tile(
                            [128, S], bf16, name=f"ot{b}", tag="ot"
                        )
                    ot = ot_tiles[b]
                    den = nrm.tile([1, 2, QG], f32, name=f"den{gi}", tag="den")
                    for h in range(HPC):
                        nc.vector.tensor_copy(
                            den[0:1, h, :], op_tiles[(gi, h)][HD : HD + 1, :]
                        )
                    rq = nrm.tile([1, 2, QG], f32, name=f"rq{gi}", tag="rq")
                    nc.vector.reciprocal_approx_fast(rq, den)
                    rbs = nrm.tile([HD, 2, QG], f32, name=f"rbs{gi}", tag="rbs")
                    nc.gpsimd.partition_broadcast(rbs, rq)
                    for h in range(HPC):
                        op = op_tiles.pop((gi, h))
                        nc.vector.tensor_mul(
                            ot[h * HD : (h + 1) * HD, qg * QG : (qg + 1) * QG],
                            op[0:HD, :],
                            rbs[:, h, :],
                        )

                return run

            def oproj_unit(gi, qt):
                b, qg = groups[gi]

                def run():
                    ot = ot_tiles[b]
                    os_ = ostage.tile([128, 1024], bf16, name=f"os{gi}_{qt}", tag="os")
                    for nh in range(2):
                        pq = ps.tile(
                            [128, 512], f32, name=f"pq{gi}{qt}{nh}", tag="op", bufs=4
                        )
                        nc.tensor.matmul(
                            pq,
                            lhsT=ot[:, qt * 128 : (qt + 1) * 128],
                            rhs=wo_sb[:, nh * 512 : (nh + 1) * 512],
                            start=True,
                            stop=True,
                        )
                        nc.vector.tensor_copy(os_[:, nh * 512 : (nh + 1) * 512], pq)
                    nc.sync.dma_start(
                        out=out[b * S + qt * 128 : b * S + (qt + 1) * 128, :],
                        in_=os_,
                    )

                return run

            # ---- pipeline stages ----
            def scores_pair(gi, kt):
                b, qg = groups[gi]
                q0 = b * S + qg * QG
                k0 = b * S + kt * 128
                sp = ps.tile([128, 2, QG], f32, name=f"sp{gi}_{kt}", tag="sp")
                for h in range(HPC):
                    hp = h * HD
                    nc.tensor.matmul(
                        sp[:, h, :],
                        lhsT=kt_sb[hp : hp + HD, k0 : k0 + 128],
                        rhs=qt_sb[hp : hp + HD, q0 : q0 + QG],
                        start=True,
                        stop=True,
                        tile_position=(hp, 0),
                    )
                et = etp.tile([128, 2, QG], bf16, name=f"et{gi}_{kt}", tag="et")
                nc.scalar.activation(et, sp, AF.Exp, scale=SCALE)
                et_tiles[(gi, kt)] = et

            def attnv_pair(gi, kt):
                b, qg = groups[gi]
                et = et_tiles.pop((gi, kt))
                for h in range(HPC):
                    if kt == 0:
                        op_tiles[(gi, h)] = ps.tile(
                            [HD + 1, QG], f32, name=f"op{gi}_{h}", tag="op", bufs=4
                        )
                    nc.tensor.matmul(
                        op_tiles[(gi, h)],
                        lhsT=v_sb[:, b * NKT + kt, h * (HD + 1) : (h + 1) * (HD + 1)],
                        rhs=et[:, h, :],
                        start=(kt == 0),
                        stop=(kt == NKT - 1),
                    )

            # ---- the slot machine ----
            from collections import deque

            fillers = deque()
            # b0 K halves 2,3 (keys kt8-15, needed by slot 8)
            fillers.append(qk_unit(False, 2))
            fillers.append(qk_unit(False, 3))
            for kt in range(NKT):            # b0 V tiles (attnV g0 from slot 4)
                fillers.append(v_unit(kt))
            fillers.append(qk_unit(True, 1))  # q for g1 (slot 16)
            fillers.append(qk_unit(True, 2))  # q for g2 (slot 32)
            fillers.append(qk_unit(True, 3))  # q for g3 (slot 48)
            for half in range(4, 8):          # b1 keys (slot 64)
                fillers.append(qk_unit(False, half))
            fillers.append(qk_unit(True, 4))  # q for g4 (slot 64)
            for kt in range(NKT, 2 * NKT):    # b1 V tiles (attnV g4 from slot 68)
                fillers.append(v_unit(kt))
            fillers.append(qk_unit(True, 5))
            fillers.append(qk_unit(True, 6))
            fillers.append(qk_unit(True, 7))

            # lead-in: q/k needed by group 0's first scores
            qk_unit(True, 0)()
            qk_unit(False, 0)()
            qk_unit(False, 1)()

            pending_attnv = deque()

            def emit_slot(slot, do_scores):
                if do_scores:
                    gi, kt = divmod(slot, NKT)
                    scores_pair(gi, kt)
                    pending_attnv.append((slot + TRAIL, gi, kt))
                while pending_attnv and pending_attnv[0][0] <= slot:
                    _, agi, akt = pending_attnv.popleft()
                    attnv_pair(agi, akt)
                    if akt == NKT - 1:
                        fillers.appendleft(norm_unit(agi))
                        b, qg = groups[agi]
                        for qt in range(qg * (QG // 128) + 3, qg * (QG // 128) - 1, -1):
                            fillers.insert(1, oproj_unit(agi, qt))
                npop = 2 if slot < 2 * NKT else 1
                for _ in range(npop):
                    if fillers:
                        fillers.popleft()()

            for slot in range(NG * NKT):
                emit_slot(slot, True)
            slot = NG * NKT
            while pending_attnv or fillers:
                emit_slot(slot, False)
                slot += 1

    nc.compile()
    return nc


def _get_prog():
    global _prog
    if _prog is None:
        _prog = _build_program()
    return _prog


def kernel(x, Wq, bq, Wk, bk, Wv, bv, Wo, bo):
    from concourse import bass_utils

    nc = _get_prog()

    xT = np.ascontiguousarray(
        np.asarray(x, dtype=np.float32).reshape(BS, D).T
    ).astype(BF16)

    in_maps = []
    for c in range(NCORES):
        cols = slice(c * CPC, (c + 1) * CPC)
        in_maps.append(
            {
                "xT": xT,
                "wq": np.ascontiguousarray(Wq[cols, :].T).astype(BF16),
                "wk": np.ascontiguousarray(Wk[cols, :].T).astype(BF16),
                "wv": np.ascontiguousarray(Wv[cols, :].T).astype(BF16),
                "wo": np.ascontiguousarray(Wo[:, cols].T).astype(BF16),
                "bq": np.asarray(bq[cols], np.float32).reshape(CPC, 1),
                "bk": np.asarray(bk[cols], np.float32).reshape(CPC, 1),
                "bv": np.asarray(bv[cols], np.float32).reshape(1, CPC).astype(BF16),
            }
        )

    res = bass_utils.run_bass_kernel_spmd(
        nc,
        in_maps,
        core_ids=list(range(NCORES)),
        trace=bool(int(os.environ.get("KERNEL_TRACE", "0"))),
    )
    kernel.last_results = res

    acc = np.zeros((BS, D), np.float64)
    for c in range(NCORES):
        acc += res.results[c]["out"].astype(np.float64)
    acc += np.asarray(bo, np.float64)[None, :]
    return acc.reshape(B, S, D).astype(np.float32)
